# revision 16
# baseline (speedup 1.0000x reference)
"""MoE (top-2 of 8 experts + shared expert) Trainium2 kernel, 8 NeuronCores.

Strategy
--------
Host (numpy): router matmul + top-2 + softmax gates (0.01% of FLOPs), token
dispatch (gather by expert), final combine (concat shared slices, scatter-add
gated expert outputs).

Device (8 cores, SPMD): core c computes
  1. expert c's FFN over the tokens routed to it (padded to capacity C)
  2. the shared-expert FFN for token slice [c*512, (c+1)*512).

All tensors are bf16 (fp32 PSUM accumulation). The routing gate g is applied
on the *output* copy (PSUM -> SBUF multiply against a broadcast gate tile), so
x is sent once and no extra device work is needed.

Loop structure keeps weights resident: every w13/w2 tile is DMA'd exactly once
and all token chunks are processed against it (the token-chunk loop is INSIDE
the weight loop; activations aT for all chunks stay in SBUF). This cuts HBM
traffic from ~400 MB/core (fp32, weights re-streamed per chunk) to ~120
MB/core, far under the PE time.

Everything is feature-major ("transposed": [feature, token]) so the
contraction dim is always the SBUF partition dim. w13 rows are interleaved
per 128-row tile (gate t at 2t, up t at 2t+1) so one weight block carries a
(gate, up) pair.
"""

import math

import ml_dtypes
import numpy as np

import concourse.bass as bass
import concourse.mybir as mybir
import concourse.tile as tile
from concourse.bass_utils import run_bass_kernel_spmd

T, D, E, F, FS, TOP_K = 4096, 2048, 8, 4096, 4096, 2
NCORES = 8
P = 128
TS = T // NCORES  # shared-expert tokens per core
DK = D // P  # 16
FT = F // P  # 32
DG = 4  # d-tiles per GEMM2 psum group (512 outputs)

F32 = mybir.dt.float32
BF16 = mybir.dt.bfloat16
BF = ml_dtypes.bfloat16


def _split_multiwaits(nc):
    """This toolchain's walrus allows at most ONE fused sem-wait per
    instruction, but TileContext's assign_waits can emit several. Split the
    extras into standalone InstEventSemaphore instructions inserted
    immediately before the owning instruction on the same engine."""
    for fn in nc.m.functions:
        for bb in fn.blocks:
            insts = list(bb.instructions)
            out = []
            changed = False
            for inst in insts:
                si = inst.sync_info
                waits = list(si.on_wait) if (si and si.on_wait) else []
                if len(waits) > 1:
                    for w in waits[:-1]:
                        out.append(
                            mybir.InstEventSemaphore(
                                name=nc.get_next_instruction_name(),
                                engine=inst.engine,
                                ins=[],
                                outs=[],
                                sync_info=mybir.SyncInfo(on_wait=[w], on_update=[]),
                            )
                        )
                    inst.sync_info = mybir.SyncInfo(
                        on_wait=[waits[-1]], on_update=list(si.on_update)
                    )
                    changed = True
                out.append(inst)
            if changed:
                bb.instructions = out


def _emit_ffn(
    nc, pools, x_d, w13_d, w2_d, out_d, g_d, chunks, fdim, last=False, win=None
):
    """One SwiGLU FFN, transposed layouts, weights streamed exactly once.

    x_d: [DK, P, n_tok] bf16. w13_d: [DK, P, 2*fdim] bf16, f-tiles interleaved
    (gate tile t at columns 2t*P, up tile t at (2t+1)*P). w2_d:
    [fdim//P, P, D] bf16. out_d: [DK, P, n_tok] f32. g_d: [P, n_tok] f32
    broadcast gate (None => plain copy out). chunks: [(c0, ct)], ct <= 512,
    relative to the token window `win` (defaults to all of x_d).
    """
    xp, wp, w2p, atp, op, gp, ps = pools
    FTl = fdim // P
    if win is None:
        win = (0, x_d.shape[2])
    w0 = win[0]
    n_tok = chunks[-1][0] + chunks[-1][1]
    silu = mybir.ActivationFunctionType.Silu

    x_ap = x_d[:].rearrange("k p c -> p k c")
    out_ap = out_d[:].rearrange("k p c -> p k c")

    def _wtile(j):
        wt = wp.tile([P, DK, P], BF16, tag="w13", name="wt")
        nc.sync.dma_start(out=wt, in_=w13_d[:][j])
        return wt

    # b=0 gate tile first, then per-k x slice tiles: the first matmul chain
    # starts as soon as the 0.5 MB gate tile + the k=0 slice land
    wtg0 = _wtile(0)
    xts = []
    for k in range(DK):
        xk = xp.tile([P, n_tok], BF16, tag="x", name=f"xt{k}")
        nc.sync.dma_start(out=xk, in_=x_ap[:, k, w0 : w0 + n_tok])
        xts.append(xk)
    wtu0 = _wtile(1)
    aT = atp.tile([P, FTl, n_tok], BF16, tag="aT", name="aT")

    # ---- GEMM1: aT[f, t] = silu(x@Wg.T) * (x@Wu.T), per (gate, up) tile pair
    for b in range(FTl):
        wtg = wtg0 if b == 0 else _wtile(2 * b)
        wtu = wtu0 if b == 0 else _wtile(2 * b + 1)
        for c0, ct in chunks:
            pt = ps.tile([P, 512], F32, tag="ps", name="ptg")
            for k in range(DK):
                nc.tensor.matmul(
                    pt[:, :ct],
                    wtg[:, k, :],
                    xts[k][:, c0 : c0 + ct],
                    start=(k == 0),
                    stop=(k == DK - 1),
                )
            nc.scalar.activation(
                out=aT[:, b, c0 : c0 + ct], in_=pt[:, :ct], func=silu
            )
        for c0, ct in chunks:
            pt = ps.tile([P, 512], F32, tag="ps", name="ptu")
            for k in range(DK):
                nc.tensor.matmul(
                    pt[:, :ct],
                    wtu[:, k, :],
                    xts[k][:, c0 : c0 + ct],
                    start=(k == 0),
                    stop=(k == DK - 1),
                )
            sl = aT[:, b, c0 : c0 + ct]
            nc.vector.tensor_mul(out=sl, in0=sl, in1=pt[:, :ct])

    # ---- GEMM2: y[d, t] = w2 @ aT, d in groups of DG tiles, w2 slice resident.
    # k-outer/gi-inner interleaves the DG psum banks so chain-start costs
    # overlap; the very last group goes gi-major so its drain pipelines into
    # the kernel tail instead of all four banks finishing at once.
    if g_d is not None:
        gt = gp.tile([P, n_tok], F32, tag="g", name="gt")
        nc.sync.dma_start(out=gt, in_=g_d[:][:, w0 : w0 + n_tok])
    for dg in range(DK // DG):
        w2ts = []
        for k in range(FTl):
            w2t = w2p.tile([P, DG * P], BF16, tag="w2", name="w2t")
            nc.sync.dma_start(
                out=w2t, in_=w2_d[:][k, :, dg * DG * P : (dg + 1) * DG * P]
            )
            w2ts.append(w2t)
        for c0, ct in chunks:
            tail = last and dg == DK // DG - 1 and (c0, ct) == chunks[-1]

            def _drain(gi, psy):
                ot = op.tile([P, 512], F32, tag="o", name="ot")
                if g_d is not None:
                    nc.vector.tensor_mul(
                        out=ot[:, :ct], in0=psy[:, :ct], in1=gt[:, c0 : c0 + ct]
                    )
                else:
                    nc.vector.tensor_copy(out=ot[:, :ct], in_=psy[:, :ct])
                nc.sync.dma_start(
                    out=out_ap[:, dg * DG + gi, w0 + c0 : w0 + c0 + ct],
                    in_=ot[:, :ct],
                )

            if tail:
                for gi in range(DG):
                    psy = ps.tile([P, 512], F32, tag="ps", name="psy")
                    for k in range(FTl):
                        nc.tensor.matmul(
                            psy[:, :ct],
                            w2ts[k][:, gi * P : (gi + 1) * P],
                            aT[:, k, c0 : c0 + ct],
                            start=(k == 0),
                            stop=(k == FTl - 1),
                        )
                    _drain(gi, psy)
            else:
                psys = [
                    ps.tile([P, 512], F32, tag="ps", name="psy")
                    for _ in range(DG)
                ]
                for k in range(FTl):
                    for gi in range(DG):
                        nc.tensor.matmul(
                            psys[gi][:, :ct],
                            w2ts[k][:, gi * P : (gi + 1) * P],
                            aT[:, k, c0 : c0 + ct],
                            start=(k == 0),
                            stop=(k == FTl - 1),
                        )
                for gi in range(DG):
                    _drain(gi, psys[gi])


def build_program(chunk_sizes):
    chunks = []
    c0 = 0
    for ct in chunk_sizes:
        chunks.append((c0, ct))
        c0 += ct
    C = c0

    nc = bass.Bass()
    xeT = nc.dram_tensor("xeT", [DK, P, C], BF16, kind="ExternalInput")
    gE = nc.dram_tensor("gE", [P, C], F32, kind="ExternalInput")
    w13T = nc.dram_tensor("w13T", [2 * F // P, P, DK, P], BF16, kind="ExternalInput")
    w2T = nc.dram_tensor("w2T", [F // P, P, D], BF16, kind="ExternalInput")
    xsT = nc.dram_tensor("xsT", [DK, P, TS], BF16, kind="ExternalInput")
    sw13T = nc.dram_tensor(
        "sw13T", [2 * FS // P, P, DK, P], BF16, kind="ExternalInput"
    )
    sw2T = nc.dram_tensor("sw2T", [FS // P, P, D], BF16, kind="ExternalInput")
    yeT = nc.dram_tensor("yeT", [DK, P, C], F32, kind="ExternalOutput")
    ysT = nc.dram_tensor("ysT", [DK, P, TS], F32, kind="ExternalOutput")

    with tile.TileContext(nc) as tc:
        with (
            tc.tile_pool(name="xp", bufs=DK) as xp,
            tc.tile_pool(name="wp", bufs=6) as wp,
            tc.tile_pool(name="w2p", bufs=FT + 4) as w2p,
            tc.tile_pool(name="atp", bufs=1) as atp,
            tc.tile_pool(name="op", bufs=3) as op,
            tc.tile_pool(name="gp", bufs=1) as gp,
            tc.tile_pool(name="ps", bufs=8, space="PSUM") as ps,
        ):
            pools = (xp, wp, w2p, atp, op, gp, ps)
            if C <= 1400:
                _emit_ffn(nc, pools, xeT, w13T, w2T, yeT, gE, chunks, F)
            else:
                # capacity too large for fully-resident aT/x: two window
                # passes (weights re-streamed once more; DMA has slack)
                h = (len(chunks) + 1) // 2
                ca, cb = chunks[:h], chunks[h:]
                wa = ca[-1][0] + ca[-1][1]
                _emit_ffn(
                    nc, pools, xeT, w13T, w2T, yeT, gE, ca, F, win=(0, wa)
                )
                _emit_ffn(
                    nc,
                    pools,
                    xeT,
                    w13T,
                    w2T,
                    yeT,
                    gE,
                    [(c0 - wa, ct) for c0, ct in cb],
                    F,
                    win=(wa, C),
                )
            _emit_ffn(
                nc, pools, xsT, sw13T, sw2T, ysT, None, [(0, TS)], FS, last=True
            )
    _split_multiwaits(nc)
    return nc


_PROG_CACHE = {}

# test harnesses may override, e.g. {"trace": True, "trace_cores": [...]}
RUN_KWARGS = {}


def _get_program(chunk_sizes):
    key = tuple(chunk_sizes)
    if key not in _PROG_CACHE:
        _PROG_CACHE[key] = build_program(key)
    return _PROG_CACHE[key]


def _interleave_w13(w13_e):
    """[2F', D] fp32 -> [2F'//P, P, DK, P] bf16, block-major: block 2t is gate
    tile t, block 2t+1 is up tile t; block[j, p, k, f] = w13[j*P+f, k*P+p].
    Each block DMA then reads one contiguous 4 KB line per partition."""
    fdim = w13_e.shape[0] // 2
    ftl = fdim // P
    wg = w13_e[:fdim].reshape(ftl, P, -1)
    wu = w13_e[fdim:].reshape(ftl, P, -1)
    wi = np.stack([wg, wu], axis=1).reshape(2 * fdim, -1)  # interleaved rows
    d = wi.shape[1]
    blk = wi.reshape(2 * ftl, P, d // P, P).transpose(0, 3, 2, 1)
    return np.ascontiguousarray(blk.astype(BF))


def kernel(x, router_DE, w13, w2, shared_w13, shared_w2):
    x = np.asarray(x, dtype=np.float32)
    router_DE = np.asarray(router_DE, dtype=np.float32)
    w13 = np.asarray(w13, dtype=np.float32)
    w2 = np.asarray(w2, dtype=np.float32)
    shared_w13 = np.asarray(shared_w13, dtype=np.float32)
    shared_w2 = np.asarray(shared_w2, dtype=np.float32)

    # ---- routing (host) ----
    logits = x @ router_DE  # [T, E]
    top_idx = np.argsort(-logits, axis=1, kind="stable")[:, :TOP_K]  # [T, K]
    top_vals = np.take_along_axis(logits, top_idx, axis=1)
    ex = np.exp(top_vals - top_vals.max(axis=1, keepdims=True))
    gates = (ex / ex.sum(axis=1, keepdims=True)).astype(np.float32)

    toks_per_e, gates_per_e = [], []
    for e in range(E):
        hit = top_idx == e  # [T, K]
        toks = np.nonzero(hit.any(axis=1))[0]
        g = (gates * hit).sum(axis=1)[toks].astype(np.float32)
        toks_per_e.append(toks)
        gates_per_e.append(g)

    max_cnt = max(len(t) for t in toks_per_e)
    n_ch = max(1, math.ceil(max_cnt / 512))
    base = max_cnt // n_ch
    chunk_sizes = tuple(
        base + (1 if i < max_cnt - base * n_ch else 0) for i in range(n_ch)
    )
    C = sum(chunk_sizes)

    # ---- host-side shard prep ----
    xTb = np.ascontiguousarray(x.T).astype(BF)  # [D, T] bf16
    sw13T = _interleave_w13(shared_w13)
    sw2T = np.ascontiguousarray(shared_w2.T.astype(BF)).reshape(FS // P, P, D)

    in_maps = []
    for c in range(NCORES):
        toks, g = toks_per_e[c], gates_per_e[c]
        cnt = len(toks)
        xe = np.zeros((D, C), BF)
        xe[:, :cnt] = xTb[:, toks]
        ge = np.zeros((P, C), np.float32)
        ge[:, :cnt] = g[None, :]
        in_maps.append(
            {
                "xeT": xe.reshape(DK, P, C),
                "gE": ge,
                "w13T": _interleave_w13(w13[c]),
                "w2T": np.ascontiguousarray(w2[c].T.astype(BF)).reshape(
                    F // P, P, D
                ),
                "xsT": np.ascontiguousarray(
                    xTb[:, c * TS : (c + 1) * TS]
                ).reshape(DK, P, TS),
                "sw13T": sw13T,
                "sw2T": sw2T,
            }
        )

    nc = _get_program(chunk_sizes)
    res = run_bass_kernel_spmd(nc, in_maps, list(range(NCORES)), **RUN_KWARGS)
    kernel.last_result = res

    # ---- combine (host) ----
    out = np.empty((T, D), np.float32)
    for c in range(NCORES):
        out[c * TS : (c + 1) * TS] = res.results[c]["ysT"].reshape(D, TS).T
    for c in range(NCORES):
        toks = toks_per_e[c]
        ye = res.results[c]["yeT"].reshape(D, C)
        out[toks] += ye[:, : len(toks)].T
    return out


# revision 19
# speedup vs baseline: 1.1906x; 1.1906x over previous
"""MoE (top-2 of 8 experts + shared expert) Trainium2 kernel, 8 NeuronCores.

Strategy
--------
Host (numpy): router matmul + top-2 + softmax gates (0.01% of FLOPs), token
dispatch (gather by expert), final combine (concat shared slices, scatter-add
gated expert outputs).

Device (8 cores, SPMD): core c computes
  1. expert c's FFN over the tokens routed to it (padded to capacity C)
  2. the shared-expert FFN for token slice [c*512, (c+1)*512).

All tensors are bf16 (fp32 PSUM accumulation). The routing gate g is applied
on the *output* copy (PSUM -> SBUF multiply against a broadcast gate tile), so
x is sent once and no extra device work is needed.

Loop structure keeps weights resident: every w13/w2 tile is DMA'd exactly once
and all token chunks are processed against it (the token-chunk loop is INSIDE
the weight loop; activations aT for all chunks stay in SBUF). This cuts HBM
traffic from ~400 MB/core (fp32, weights re-streamed per chunk) to ~120
MB/core, far under the PE time.

Everything is feature-major ("transposed": [feature, token]) so the
contraction dim is always the SBUF partition dim. w13 rows are interleaved
per 128-row tile (gate t at 2t, up t at 2t+1) so one weight block carries a
(gate, up) pair.
"""

import math

import ml_dtypes
import numpy as np

import concourse.bass as bass
import concourse.mybir as mybir
import concourse.tile as tile
from concourse.bass_utils import run_bass_kernel_spmd

T, D, E, F, FS, TOP_K = 4096, 2048, 8, 4096, 4096, 2
NCORES = 8
P = 128
TS = T // NCORES  # shared-expert tokens per core
DK = D // P  # 16
FT = F // P  # 32
DG = 4  # d-tiles per GEMM2 psum group (512 outputs)

F32 = mybir.dt.float32
BF16 = mybir.dt.bfloat16
BF = ml_dtypes.bfloat16


def _split_multiwaits(nc):
    """This toolchain's walrus allows at most ONE fused sem-wait per
    instruction, but TileContext's assign_waits can emit several. Split the
    extras into standalone InstEventSemaphore instructions inserted
    immediately before the owning instruction on the same engine."""
    for fn in nc.m.functions:
        for bb in fn.blocks:
            insts = list(bb.instructions)
            out = []
            changed = False
            for inst in insts:
                si = inst.sync_info
                waits = list(si.on_wait) if (si and si.on_wait) else []
                if len(waits) > 1:
                    for w in waits[:-1]:
                        out.append(
                            mybir.InstEventSemaphore(
                                name=nc.get_next_instruction_name(),
                                engine=inst.engine,
                                ins=[],
                                outs=[],
                                sync_info=mybir.SyncInfo(on_wait=[w], on_update=[]),
                            )
                        )
                    inst.sync_info = mybir.SyncInfo(
                        on_wait=[waits[-1]], on_update=list(si.on_update)
                    )
                    changed = True
                out.append(inst)
            if changed:
                bb.instructions = out


def _emit_ffn(
    nc, pools, x_d, w13_d, w2_d, out_d, g_d, chunks, fdim, last=False, win=None
):
    """One SwiGLU FFN, transposed layouts, weights streamed exactly once.

    x_d: [DK, P, n_tok] bf16. w13_d: [DK, P, 2*fdim] bf16, f-tiles interleaved
    (gate tile t at columns 2t*P, up tile t at (2t+1)*P). w2_d:
    [fdim//P, P, D] bf16. out_d: [DK, P, n_tok] f32. g_d: [P, n_tok] f32
    broadcast gate (None => plain copy out). chunks: [(c0, ct)], ct <= 512,
    relative to the token window `win` (defaults to all of x_d).
    """
    xp, wp, w2p, atp, op, gp, ps = pools
    FTl = fdim // P
    if win is None:
        win = (0, x_d.shape[2])
    w0 = win[0]
    n_tok = chunks[-1][0] + chunks[-1][1]
    silu = mybir.ActivationFunctionType.Silu

    x_ap = x_d[:].rearrange("k p c -> p k c")
    w13_ap = w13_d[:].rearrange("k p f -> p k f")
    out_ap = out_d[:].rearrange("k p c -> p k c")

    # first weight block, then per-k x slice tiles: the b=0 matmul chain can
    # start as soon as wt0 + the k=0 slice land instead of the full x DMA
    wt0 = wp.tile([P, DK, 2 * P], BF16, tag="w13", name="wt")
    nc.sync.dma_start(out=wt0, in_=w13_ap[:, :, 0 : 2 * P])
    xts = []
    for k in range(DK):
        xk = xp.tile([P, n_tok], BF16, tag="x", name=f"xt{k}")
        nc.sync.dma_start(out=xk, in_=x_ap[:, k, w0 : w0 + n_tok])
        xts.append(xk)
    aT = atp.tile([P, FTl, n_tok], BF16, tag="aT", name="aT")

    # ---- GEMM1: aT[f, t] = silu(x@Wg.T) * (x@Wu.T), per interleaved block
    for b in range(FTl):
        if b == 0:
            wt = wt0
        else:
            wt = wp.tile([P, DK, 2 * P], BF16, tag="w13", name="wt")
            nc.sync.dma_start(
                out=wt, in_=w13_ap[:, :, 2 * b * P : 2 * (b + 1) * P]
            )
        for c0, ct in chunks:
            pt = ps.tile([P, 512], F32, tag="ps", name="ptg")
            for k in range(DK):
                nc.tensor.matmul(
                    pt[:, :ct],
                    wt[:, k, 0:P],
                    xts[k][:, c0 : c0 + ct],
                    start=(k == 0),
                    stop=(k == DK - 1),
                )
            nc.scalar.activation(
                out=aT[:, b, c0 : c0 + ct], in_=pt[:, :ct], func=silu
            )
        for c0, ct in chunks:
            pt = ps.tile([P, 512], F32, tag="ps", name="ptu")
            for k in range(DK):
                nc.tensor.matmul(
                    pt[:, :ct],
                    wt[:, k, P : 2 * P],
                    xts[k][:, c0 : c0 + ct],
                    start=(k == 0),
                    stop=(k == DK - 1),
                )
            sl = aT[:, b, c0 : c0 + ct]
            nc.vector.tensor_mul(out=sl, in0=sl, in1=pt[:, :ct])

    # ---- GEMM2: y[d, t] = w2 @ aT, d in groups of DG tiles, w2 slice resident.
    # k-outer/gi-inner interleaves the DG psum banks so chain-start costs
    # overlap; the very last group goes gi-major so its drain pipelines into
    # the kernel tail instead of all four banks finishing at once.
    if g_d is not None:
        gt = gp.tile([P, n_tok], F32, tag="g", name="gt")
        nc.sync.dma_start(out=gt, in_=g_d[:][:, w0 : w0 + n_tok])
    for dg in range(DK // DG):
        w2ts = []
        for k in range(FTl):
            w2t = w2p.tile([P, DG * P], BF16, tag="w2", name="w2t")
            nc.sync.dma_start(
                out=w2t, in_=w2_d[:][k, :, dg * DG * P : (dg + 1) * DG * P]
            )
            w2ts.append(w2t)
        for c0, ct in chunks:
            tail = last and dg == DK // DG - 1 and (c0, ct) == chunks[-1]

            def _drain(gi, psy):
                ot = op.tile([P, 512], F32, tag="o", name="ot")
                if g_d is not None:
                    nc.vector.tensor_mul(
                        out=ot[:, :ct], in0=psy[:, :ct], in1=gt[:, c0 : c0 + ct]
                    )
                else:
                    nc.vector.tensor_copy(out=ot[:, :ct], in_=psy[:, :ct])
                nc.sync.dma_start(
                    out=out_ap[:, dg * DG + gi, w0 + c0 : w0 + c0 + ct],
                    in_=ot[:, :ct],
                )

            if tail:
                for gi in range(DG):
                    psy = ps.tile([P, 512], F32, tag="ps", name="psy")
                    for k in range(FTl):
                        nc.tensor.matmul(
                            psy[:, :ct],
                            w2ts[k][:, gi * P : (gi + 1) * P],
                            aT[:, k, c0 : c0 + ct],
                            start=(k == 0),
                            stop=(k == FTl - 1),
                        )
                    _drain(gi, psy)
            else:
                psys = [
                    ps.tile([P, 512], F32, tag="ps", name="psy")
                    for _ in range(DG)
                ]
                for k in range(FTl):
                    for gi in range(DG):
                        nc.tensor.matmul(
                            psys[gi][:, :ct],
                            w2ts[k][:, gi * P : (gi + 1) * P],
                            aT[:, k, c0 : c0 + ct],
                            start=(k == 0),
                            stop=(k == FTl - 1),
                        )
                for gi in range(DG):
                    _drain(gi, psys[gi])


def build_program(chunk_sizes):
    chunks = []
    c0 = 0
    for ct in chunk_sizes:
        chunks.append((c0, ct))
        c0 += ct
    C = c0

    nc = bass.Bass()
    xeT = nc.dram_tensor("xeT", [DK, P, C], BF16, kind="ExternalInput")
    gE = nc.dram_tensor("gE", [P, C], F32, kind="ExternalInput")
    w13T = nc.dram_tensor("w13T", [DK, P, 2 * F], BF16, kind="ExternalInput")
    w2T = nc.dram_tensor("w2T", [F // P, P, D], BF16, kind="ExternalInput")
    xsT = nc.dram_tensor("xsT", [DK, P, TS], BF16, kind="ExternalInput")
    sw13T = nc.dram_tensor("sw13T", [DK, P, 2 * FS], BF16, kind="ExternalInput")
    sw2T = nc.dram_tensor("sw2T", [FS // P, P, D], BF16, kind="ExternalInput")
    yeT = nc.dram_tensor("yeT", [DK, P, C], F32, kind="ExternalOutput")
    ysT = nc.dram_tensor("ysT", [DK, P, TS], F32, kind="ExternalOutput")

    with tile.TileContext(nc) as tc:
        with (
            tc.tile_pool(name="xp", bufs=DK) as xp,
            tc.tile_pool(name="wp", bufs=3) as wp,
            tc.tile_pool(name="w2p", bufs=FT + 4) as w2p,
            tc.tile_pool(name="atp", bufs=1) as atp,
            tc.tile_pool(name="op", bufs=3) as op,
            tc.tile_pool(name="gp", bufs=1) as gp,
            tc.tile_pool(name="ps", bufs=8, space="PSUM") as ps,
        ):
            pools = (xp, wp, w2p, atp, op, gp, ps)
            if C <= 1400:
                _emit_ffn(nc, pools, xeT, w13T, w2T, yeT, gE, chunks, F)
            else:
                # capacity too large for fully-resident aT/x: two window
                # passes (weights re-streamed once more; DMA has slack)
                h = (len(chunks) + 1) // 2
                ca, cb = chunks[:h], chunks[h:]
                wa = ca[-1][0] + ca[-1][1]
                _emit_ffn(
                    nc, pools, xeT, w13T, w2T, yeT, gE, ca, F, win=(0, wa)
                )
                _emit_ffn(
                    nc,
                    pools,
                    xeT,
                    w13T,
                    w2T,
                    yeT,
                    gE,
                    [(c0 - wa, ct) for c0, ct in cb],
                    F,
                    win=(wa, C),
                )
            _emit_ffn(
                nc, pools, xsT, sw13T, sw2T, ysT, None, [(0, TS)], FS, last=True
            )
    _split_multiwaits(nc)
    return nc


_PROG_CACHE = {}

# test harnesses may override, e.g. {"trace": True, "trace_cores": [...]}
RUN_KWARGS = {}


def _get_program(chunk_sizes):
    key = tuple(chunk_sizes)
    if key not in _PROG_CACHE:
        _PROG_CACHE[key] = build_program(key)
    return _PROG_CACHE[key]


def _interleave_w13(w13_e):
    """[2F', D] fp32 -> [DK, P, 2F'] bf16 with (gate, up) 128-row tiles
    interleaved along the output feature axis."""
    fdim = w13_e.shape[0] // 2
    ftl = fdim // P
    wg = w13_e[:fdim].reshape(ftl, P, -1)
    wu = w13_e[fdim:].reshape(ftl, P, -1)
    wi = np.stack([wg, wu], axis=1).reshape(2 * fdim, -1)  # interleaved rows
    return np.ascontiguousarray(wi.T.astype(BF)).reshape(DK, P, 2 * fdim)


def kernel(x, router_DE, w13, w2, shared_w13, shared_w2):
    x = np.asarray(x, dtype=np.float32)
    router_DE = np.asarray(router_DE, dtype=np.float32)
    w13 = np.asarray(w13, dtype=np.float32)
    w2 = np.asarray(w2, dtype=np.float32)
    shared_w13 = np.asarray(shared_w13, dtype=np.float32)
    shared_w2 = np.asarray(shared_w2, dtype=np.float32)

    # ---- routing (host) ----
    logits = x @ router_DE  # [T, E]
    top_idx = np.argsort(-logits, axis=1, kind="stable")[:, :TOP_K]  # [T, K]
    top_vals = np.take_along_axis(logits, top_idx, axis=1)
    ex = np.exp(top_vals - top_vals.max(axis=1, keepdims=True))
    gates = (ex / ex.sum(axis=1, keepdims=True)).astype(np.float32)

    toks_per_e, gates_per_e = [], []
    for e in range(E):
        hit = top_idx == e  # [T, K]
        toks = np.nonzero(hit.any(axis=1))[0]
        g = (gates * hit).sum(axis=1)[toks].astype(np.float32)
        toks_per_e.append(toks)
        gates_per_e.append(g)

    max_cnt = max(len(t) for t in toks_per_e)
    n_ch = max(1, math.ceil(max_cnt / 512))
    base = max_cnt // n_ch
    chunk_sizes = tuple(
        base + (1 if i < max_cnt - base * n_ch else 0) for i in range(n_ch)
    )
    C = sum(chunk_sizes)

    # ---- host-side shard prep ----
    xTb = np.ascontiguousarray(x.T).astype(BF)  # [D, T] bf16
    sw13T = _interleave_w13(shared_w13)
    sw2T = np.ascontiguousarray(shared_w2.T.astype(BF)).reshape(FS // P, P, D)

    in_maps = []
    for c in range(NCORES):
        toks, g = toks_per_e[c], gates_per_e[c]
        cnt = len(toks)
        xe = np.zeros((D, C), BF)
        xe[:, :cnt] = xTb[:, toks]
        ge = np.zeros((P, C), np.float32)
        ge[:, :cnt] = g[None, :]
        in_maps.append(
            {
                "xeT": xe.reshape(DK, P, C),
                "gE": ge,
                "w13T": _interleave_w13(w13[c]),
                "w2T": np.ascontiguousarray(w2[c].T.astype(BF)).reshape(
                    F // P, P, D
                ),
                "xsT": np.ascontiguousarray(
                    xTb[:, c * TS : (c + 1) * TS]
                ).reshape(DK, P, TS),
                "sw13T": sw13T,
                "sw2T": sw2T,
            }
        )

    nc = _get_program(chunk_sizes)
    res = run_bass_kernel_spmd(nc, in_maps, list(range(NCORES)), **RUN_KWARGS)
    kernel.last_result = res

    # ---- combine (host) ----
    out = np.empty((T, D), np.float32)
    for c in range(NCORES):
        out[c * TS : (c + 1) * TS] = res.results[c]["ysT"].reshape(D, TS).T
    for c in range(NCORES):
        toks = toks_per_e[c]
        ye = res.results[c]["yeT"].reshape(D, C)
        out[toks] += ye[:, : len(toks)].T
    return out
